# revision 20
# baseline (speedup 1.0000x reference)
"""Trainium2 Bass kernel for nn_PrefrontalCortex (scatter_memory).

Sharding (8 cores, tensor-parallel / reduction-dim sharding):
  - read_W row-sharded: core c computes read_key[128c:128c+128] locally (fp32).
  - memory column-sharded (MEM_DIM): core c holds memory[:, 128c:128c+128]
    in two layouts (transposed for sim, natural for read_vec/update), fp32.
    sim partial = mem_shard @ read_key_slice  -> AllReduce #1 (16 KB).
    softmax + usage_new computed fully on every core; read_vec slice local.
  - W_ih / W_hh / out_W column-sharded (contraction dim), stored as bf16
    hi/lo pairs; GEMVs run 3 products (Whi*xhi + Whi*xlo + Wlo*xhi) into
    fp32 PSUM for near-fp32 accuracy at bf16 PE speed. head_W is plain bf16
    (its outputs - erase/add/write-logits - are insensitive).
    Partial GEMV outputs -> AllReduce #2 (gates, 32 KB) and
    AllReduce #3 (head params + out merged, 37 KB).
  - memory erase/add update is local per column shard; new_memory written
    sharded, gathered on host.

GEMVs use stationary 128x128 W^T blocks on the PE (outputs land col-major
across partitions). DMA rings: SP ring streams all weights in priority
order; ACT ring carries only latency-critical small transfers (collective
bounces, small outputs).
"""

import numpy as np

C = 8                       # cores
P = 128
H = 2048                    # hidden
IN = 2048                   # input dim
S = 4096                    # mem slots
M = 1024                    # mem dim
OUT = 2048
G4 = 4 * H                  # 8192
HEAD = 3 * M + S + 1        # 7169
MH = 57                     # head m-chunks (7296 padded)
HEAD_PAD = MH * P           # 7296
MG = G4 // P                # 64
MO = OUT // P               # 16
SB = S // P                 # 32
KH = H // P                 # 16  (h k-tiles, full)
XC = IN // C                # 256 per-core x cols (2 k-tiles)
HC = H // C                 # 256 per-core h cols
MC = M // C                 # 128 per-core mem cols (1 k-tile)

# smalls column layout (fp32)
_S_HK = 0          # [16] h_prev full, col-major (fp32, for read_key GEMV)
_S_RB = 16         # [1]  read_b slice
_S_GB = 17         # [64] b_ih + b_hh, col-major
_S_HB = 81         # [57] head_b padded, col-major
_S_OB = 138        # [16] out_b col-major
_S_CP = 154        # [16] c_prev col-major
_S_US = 170        # [32] usage col-major (slot)
_S_ID = 202        # [128] identity
NS = 330

_CACHE = {}


def _pe_layout(w):
    """w: [rows_out, cols_k] (rows % 128 == 0, cols % 128 == 0)
    -> [128, nk, nm, 128] with arr[p, k, m, j] = w[m*128 + j, k*128 + p]."""
    nm = w.shape[0] // P
    nk = w.shape[1] // P
    return np.ascontiguousarray(
        w.reshape(nm, P, nk, P).transpose(3, 2, 0, 1)
    ).astype(np.float32)


def _cm(v):
    """vector [n*128] -> col-major [128, n]: out[p, m] = v[m*128 + p]"""
    n = v.shape[0] // P
    return np.ascontiguousarray(v.reshape(n, P).T).astype(np.float32)


def _hilo(a):
    """f32 array -> (hi, lo) bf16 pair with hi + lo ~= a."""
    import ml_dtypes
    hi = a.astype(ml_dtypes.bfloat16)
    lo = (a - hi.astype(np.float32)).astype(ml_dtypes.bfloat16)
    return hi, lo


def _pe_hilo(w):
    return _hilo(_pe_layout(w))


def _build():
    if "nc" in _CACHE:
        return _CACHE["nc"]

    import concourse.bass as bass
    import concourse.bacc as bacc
    import concourse.mybir as mybir
    import concourse.tile as tile
    import bass_rust

    dt = mybir.dt.float32
    bf = mybir.dt.bfloat16
    A = mybir.AluOpType
    F = mybir.ActivationFunctionType
    ROp = bass_rust.ReduceOp

    nc = bacc.Bacc("TRN2", target_bir_lowering=False, debug=False, num_devices=C)

    # ---- I/O ----
    d_wihx = [nc.dram_tensor(f"wihx_{s}", [P, 2, MG, P], bf,
                             kind="ExternalInput") for s in ("hi", "lo")]
    d_whh = [nc.dram_tensor(f"whh_{s}", [P, 2, MG, P], bf,
                            kind="ExternalInput") for s in ("hi", "lo")]
    d_wihr = [nc.dram_tensor(f"wihr_{s}", [P, MG, P], bf,
                             kind="ExternalInput") for s in ("hi", "lo")]
    d_headw = nc.dram_tensor("headw", [P, 2, MH, P], bf, kind="ExternalInput")
    d_outwh = [nc.dram_tensor(f"outwh_{s}", [P, 2, MO, P], bf,
                              kind="ExternalInput") for s in ("hi", "lo")]
    d_outwr = [nc.dram_tensor(f"outwr_{s}", [P, MO, P], bf,
                              kind="ExternalInput") for s in ("hi", "lo")]
    d_rwt = nc.dram_tensor("rwt", [P, KH, P], dt, kind="ExternalInput")
    d_memT = nc.dram_tensor("memT", [P, S], dt, kind="ExternalInput")
    d_memN = [nc.dram_tensor(f"memN_{s}", [P, SB, P], bf,
                             kind="ExternalInput") for s in ("hi", "lo")]
    d_smalls = nc.dram_tensor("smalls", [P, NS], dt, kind="ExternalInput")
    d_smallsb = nc.dram_tensor("smallsb", [P, 8], bf, kind="ExternalInput")

    d_onm = nc.dram_tensor("o_new_mem", [P, SB, P], dt, kind="ExternalOutput")
    d_ous = nc.dram_tensor("o_usage", [P, SB], dt, kind="ExternalOutput")
    d_oout = nc.dram_tensor("o_out", [P, MO], dt, kind="ExternalOutput")
    d_oh = nc.dram_tensor("o_h", [P, MO], dt, kind="ExternalOutput")
    d_oc = nc.dram_tensor("o_c", [P, MO], dt, kind="ExternalOutput")

    RG = [list(range(C))]

    from contextlib import ExitStack

    with tile.TileContext(nc) as tc, ExitStack() as ctx:
        big = ctx.enter_context(tc.tile_pool(name="big", bufs=1))
        stream = ctx.enter_context(tc.tile_pool(name="stream", bufs=4))
        work = ctx.enter_context(tc.tile_pool(name="work", bufs=1))
        upd = ctx.enter_context(tc.tile_pool(name="upd", bufs=4))
        ps = ctx.enter_context(tc.tile_pool(name="ps", bufs=1, space="PSUM"))
        dram = ctx.enter_context(tc.tile_pool(name="dram", bufs=1, space="DRAM"))

        # ============ ACT ring: tiny, latency-critical ============
        smalls = work.tile([P, NS], dt)
        nc.scalar.dma_start(smalls[:, :], d_smalls[:, :])
        smallsb = work.tile([P, 8], bf)
        nc.scalar.dma_start(smallsb[:, :], d_smallsb[:, :])

        # read-path tensors on the ACT ring so they beat the weight flood
        rwt = big.tile([P, KH, P], dt)
        nc.scalar.dma_start(rwt[:, :, :], d_rwt[:, :, :])
        # ============ SP ring: all weights, in priority order ============
        memT = big.tile([P, S], dt)
        nc.sync.dma_start(memT[:, :], d_memT[:, :])
        memN = [big.tile([P, SB, P], bf, name=f"memN_{s}") for s in (0, 1)]
        for s in (0, 1):
            nc.sync.dma_start(memN[s][:, :, :], d_memN[s][:, :, :])
        memF = big.tile([P, SB, P], dt)
        wihr = [big.tile([P, MG, P], bf, name=f"wihr_{s}") for s in (0, 1)]
        for s in (0, 1):
            nc.sync.dma_start(wihr[s][:, :, :], d_wihr[s][:, :, :])
        outwr = [big.tile([P, MO, P], bf, name=f"outwr_{s}") for s in (0, 1)]
        for s in (0, 1):
            nc.sync.dma_start(outwr[s][:, :, :], d_outwr[s][:, :, :])

        # streamed gates weights: per m-half, per k-tile, hi then lo
        chunks = []
        chunk_src = []  # (dram_pair, k, hi/lo)
        for k in (0, 1):
            for s in (0, 1):
                chunk_src.append((d_wihx, k, s))
        for k in (0, 1):
            for s in (0, 1):
                chunk_src.append((d_whh, k, s))
        for mh in (0, 1):
            for (srcp, k, s) in chunk_src:
                t = stream.tile([P, 32, P], bf, tag="wch",
                                name=f"wch_{mh}_{k}_{s}_{srcp[s].name}")
                nc.sync.dma_start(t[:, :, :],
                                  srcp[s][:, k, mh * 32:(mh + 1) * 32, :])
                chunks.append(t)

        headw = big.tile([P, 2, MH, P], bf)
        nc.sync.dma_start(headw[:, 0, :, :], d_headw[:, 0, :, :])
        nc.sync.dma_start(headw[:, 1, :, :], d_headw[:, 1, :, :])
        outwh = [big.tile([P, 2, MO, P], bf, name=f"outwh_{s}")
                 for s in (0, 1)]
        for s in (0, 1):
            nc.sync.dma_start(outwh[s][:, :, :, :], d_outwh[s][:, :, :, :])

        # smalls views
        hk = smalls[:, _S_HK:_S_HK + KH]
        rb = smalls[:, _S_RB:_S_RB + 1]
        gb = smalls[:, _S_GB:_S_GB + MG]
        hb = smalls[:, _S_HB:_S_HB + MH]
        ob = smalls[:, _S_OB:_S_OB + MO]
        cp = smalls[:, _S_CP:_S_CP + MO]
        usg = smalls[:, _S_US:_S_US + SB]
        ident = smalls[:, _S_ID:_S_ID + P]
        x2h = smallsb[:, 0:2]
        x2l = smallsb[:, 2:4]
        h2h = smallsb[:, 4:6]
        h2l = smallsb[:, 6:8]

        # PSUM tiles
        rk_ps = ps.tile([P, 1], dt)
        sim_ps = ps.tile([P, SB], dt)
        gates_ps = ps.tile([P, MG], dt)
        rv_ps = ps.tile([P, 1], dt)
        par_ps = ps.tile([P, MH], dt)
        out_ps = ps.tile([P, MO], dt)
        ea_ps = ps.tile([64, P], dt)

        # DRAM bounce buffers for collectives
        ar1_i = dram.tile([P, SB], dt)
        ar1_o = dram.tile([P, SB], dt)
        ar2_i = dram.tile([P, MG], dt)
        ar2_o = dram.tile([P, MG], dt)
        ar3_i = dram.tile([P, MH + MO], dt)
        ar3_o = dram.tile([P, MH + MO], dt)

        MM = nc.tensor.matmul
        pid = nc.partition_id(engines=(mybir.EngineType.DVE,))

        # ============ read path (fp32) ============
        for k in range(KH):
            MM(rk_ps[:, 0:1], rwt[:, k, :], hk[:, k:k + 1],
               start=(k == 0), stop=(k == KH - 1), skip_group_check=True)
        rk = work.tile([P, 1], dt)
        nc.scalar.activation(rk[:, :], rk_ps[:, :], F.Tanh, bias=rb)

        for sb in range(SB):
            MM(sim_ps[:, sb:sb + 1], memT[:, sb * P:(sb + 1) * P], rk[:, 0:1],
               start=(sb == 0), stop=True, skip_group_check=True)
        sim_sb = work.tile([P, SB], dt)
        nc.vector.tensor_copy(sim_sb[:, :], sim_ps[:, :])
        nc.scalar.dma_start(ar1_i[:, :], sim_sb[:, :])
        nc.gpsimd.collective_compute(
            "AllReduce", A.add, replica_groups=RG,
            ins=[ar1_i[:, :].opt()], outs=[ar1_o[:, :].opt()])
        simf = work.tile([P, SB], dt)
        nc.scalar.dma_start(simf[:, :], ar1_o[:, :])

        # softmax(simf) -> rw [P, SB] col-major over slots
        mx = work.tile([P, 1], dt)
        nc.vector.tensor_reduce(mx[:, :], simf[:, :],
                                axis=mybir.AxisListType.X, op=A.max)
        mxa = work.tile([P, 1], dt)
        nc.gpsimd.partition_all_reduce(mxa[:, :], mx[:, :], 128, ROp.max)
        nmx = work.tile([P, 1], dt)
        nc.vector.tensor_scalar_mul(nmx[:, :], mxa[:, :], -1.0)
        ex = work.tile([P, SB], dt)
        den_p = work.tile([P, 1], dt)
        nc.scalar.activation(ex[:, :], simf[:, :], F.Exp, bias=nmx,
                             accum_out=den_p[:, :])
        den = work.tile([P, 1], dt)
        nc.gpsimd.partition_all_reduce(den[:, :], den_p[:, :], 128, ROp.add)
        rden = work.tile([P, 1], dt)
        nc.vector.reciprocal(rden[:, :], den[:, :])
        rw = work.tile([P, SB], dt)
        nc.vector.tensor_scalar_mul(rw[:, :], ex[:, :], rden[:, :])

        # usage_new (full, identical on every core)
        un = work.tile([P, SB], dt)
        nc.vector.tensor_add(un[:, :], usg, rw[:, :])
        nc.sync.dma_start(d_ous[:, :], un[:, :])
        mask01 = work.tile([P, SB], dt)
        nc.vector.tensor_scalar(mask01[:, :], un[:, :], 0.1, None, op0=A.is_ge)

        # ============ gates GEMV xh part (bf16 hi/lo partials) ============
        for mh in (0, 1):
            for i, (srcp, k, s) in enumerate(chunk_src):
                t = chunks[mh * 8 + i]
                rh, rl = (x2h, x2l) if srcp is d_wihx else (h2h, h2l)
                for m in range(32):
                    gm = mh * 32 + m
                    last_gates_mm = MM(
                        gates_ps[:, gm:gm + 1], t[:, m, :], rh[:, k:k + 1],
                        start=(mh == 0 and i == 0 and m == 0), stop=False,
                        skip_group_check=True)
                    if s == 0:  # hi weights also multiply lo rhs
                        last_gates_mm = MM(
                            gates_ps[:, gm:gm + 1], t[:, m, :], rl[:, k:k + 1],
                            start=False, stop=False, skip_group_check=True)

        # reconstruct f32 memory for the update (hi + lo), off-chain
        for sb in range(SB):
            nc.vector.tensor_add(memF[:, sb, :], memN[0][:, sb, :],
                                 memN[1][:, sb, :])

        # rw hi/lo split for the bf16 read_vec GEMV
        rw_h = work.tile([P, SB], bf)
        nc.vector.tensor_copy(rw_h[:, :], rw[:, :])
        rw_hf = work.tile([P, SB], dt)
        nc.vector.tensor_copy(rw_hf[:, :], rw_h[:, :])
        rw_lf = work.tile([P, SB], dt)
        nc.vector.tensor_sub(rw_lf[:, :], rw[:, :], rw_hf[:, :])
        rw_l = work.tile([P, SB], bf)
        nc.vector.tensor_copy(rw_l[:, :], rw_lf[:, :])

        # ============ read_vec (bf16 hi/lo, after AR1) ============
        from concourse.tile_rust import add_dep_helper
        first_rv = None
        for sb in range(SB):
            i1 = MM(rv_ps[:, 0:1], memN[0][:, sb, :], rw_h[:, sb:sb + 1],
                    start=(sb == 0), stop=False, skip_group_check=True)
            if first_rv is None:
                first_rv = i1
                add_dep_helper(last_gates_mm.ins, first_rv.ins, sync=True,
                               reason="keep rv after streamed gates-xh")
            MM(rv_ps[:, 0:1], memN[0][:, sb, :], rw_l[:, sb:sb + 1],
               start=False, stop=False, skip_group_check=True)
            MM(rv_ps[:, 0:1], memN[1][:, sb, :], rw_h[:, sb:sb + 1],
               start=False, stop=(sb == SB - 1), skip_group_check=True)
        rv = work.tile([P, 1], dt)
        nc.vector.tensor_copy(rv[:, :], rv_ps[:, :])
        rv_h = work.tile([P, 1], bf)
        nc.vector.tensor_copy(rv_h[:, :], rv[:, :])
        rv_hf = work.tile([P, 1], dt)
        nc.vector.tensor_copy(rv_hf[:, :], rv_h[:, :])
        rv_lf = work.tile([P, 1], dt)
        nc.vector.tensor_sub(rv_lf[:, :], rv[:, :], rv_hf[:, :])
        rv_l = work.tile([P, 1], bf)
        nc.vector.tensor_copy(rv_l[:, :], rv_lf[:, :])

        # mask reconstructed memory in place (DVE; keep the ACT ring free)
        for sb in range(SB):
            nc.vector.tensor_scalar_mul(memF[:, sb, :], memF[:, sb, :],
                                        mask01[:, sb:sb + 1])

        # read_vec contribution closes the gates accumulation
        for m in range(MG):
            MM(gates_ps[:, m:m + 1], wihr[0][:, m, :], rv_h[:, 0:1],
               start=False, stop=False, skip_group_check=True)
            MM(gates_ps[:, m:m + 1], wihr[0][:, m, :], rv_l[:, 0:1],
               start=False, stop=False, skip_group_check=True)
            MM(gates_ps[:, m:m + 1], wihr[1][:, m, :], rv_h[:, 0:1],
               start=False, stop=(m == MG - 1), skip_group_check=True)

        gpart = work.tile([P, MG], dt)
        nc.vector.tensor_copy(gpart[:, :], gates_ps[:, :])
        nc.sync.dma_start(ar2_i[:, :], gpart[:, :])
        nc.gpsimd.collective_compute(
            "AllReduce", A.add, replica_groups=RG,
            ins=[ar2_i[:, :].opt()], outs=[ar2_o[:, :].opt()])
        gts = work.tile([P, MG], dt)
        nc.sync.dma_start(gts[:, :], ar2_o[:, :])
        nc.vector.tensor_add(gts[:, :], gts[:, :], gb)

        # out: rv part can start as soon as rv is ready
        for m in range(MO):
            MM(out_ps[:, m:m + 1], outwr[0][:, m, :], rv_h[:, 0:1],
               start=(m == 0), stop=False, skip_group_check=True)
            MM(out_ps[:, m:m + 1], outwr[0][:, m, :], rv_l[:, 0:1],
               start=False, stop=False, skip_group_check=True)
            MM(out_ps[:, m:m + 1], outwr[1][:, m, :], rv_h[:, 0:1],
               start=False, stop=False, skip_group_check=True)

        # ============ LSTM cell (full, identical on every core) ============
        gi = gts[:, 0:16]
        gf = gts[:, 16:32]
        gg = gts[:, 32:48]
        go = gts[:, 48:64]
        si = work.tile([P, MO], dt)
        sf = work.tile([P, MO], dt)
        tg = work.tile([P, MO], dt)
        so = work.tile([P, MO], dt)
        nc.scalar.activation(si[:, :], gi, F.Sigmoid)
        nc.scalar.activation(sf[:, :], gf, F.Sigmoid)
        nc.scalar.activation(so[:, :], go, F.Sigmoid)
        nc.scalar.activation(tg[:, :], gg, F.Tanh)
        t1 = work.tile([P, MO], dt)
        nc.vector.tensor_mul(t1[:, :], si[:, :], tg[:, :])
        t2 = work.tile([P, MO], dt)
        nc.vector.tensor_mul(t2[:, :], sf[:, :], cp)
        c_t = work.tile([P, MO], dt)
        nc.vector.tensor_add(c_t[:, :], t1[:, :], t2[:, :])
        tc_ = work.tile([P, MO], dt)
        nc.scalar.activation(tc_[:, :], c_t[:, :], F.Tanh)
        h_t = work.tile([P, MO], dt)
        nc.vector.tensor_mul(h_t[:, :], so[:, :], tc_[:, :])
        nc.sync.dma_start(d_oc[:, :], c_t[:, :])
        nc.sync.dma_start(d_oh[:, :], h_t[:, :])

        # this core's two h k-tiles (dynamic by core id), bf16 hi/lo
        hloc = work.tile([P, 2], dt)
        nc.vector.tensor_copy(hloc[:, :], h_t[:, bass.ts(pid, 2)])
        hloc_h = work.tile([P, 2], bf)
        nc.vector.tensor_copy(hloc_h[:, :], hloc[:, :])
        hloc_hf = work.tile([P, 2], dt)
        nc.vector.tensor_copy(hloc_hf[:, :], hloc_h[:, :])
        hloc_lf = work.tile([P, 2], dt)
        nc.vector.tensor_sub(hloc_lf[:, :], hloc[:, :], hloc_hf[:, :])
        hloc_l = work.tile([P, 2], bf)
        nc.vector.tensor_copy(hloc_l[:, :], hloc_lf[:, :])

        # ============ head + out h-part GEMVs (partial) ============
        for kk in (0, 1):
            for m in range(MO):
                MM(out_ps[:, m:m + 1], outwh[0][:, kk, m, :],
                   hloc_h[:, kk:kk + 1],
                   start=False, stop=False, skip_group_check=True)
                MM(out_ps[:, m:m + 1], outwh[0][:, kk, m, :],
                   hloc_l[:, kk:kk + 1],
                   start=False, stop=False, skip_group_check=True)
                MM(out_ps[:, m:m + 1], outwh[1][:, kk, m, :],
                   hloc_h[:, kk:kk + 1],
                   start=False, stop=(kk == 1 and m == MO - 1),
                   skip_group_check=True)
        for kk in (0, 1):
            for m in range(MH):
                MM(par_ps[:, m:m + 1], headw[:, kk, m, :],
                   hloc_h[:, kk:kk + 1],
                   start=(kk == 0 and m == 0), stop=False,
                   skip_group_check=True)
                MM(par_ps[:, m:m + 1], headw[:, kk, m, :],
                   hloc_l[:, kk:kk + 1],
                   start=False, stop=(kk == 1 and m == MH - 1),
                   skip_group_check=True)

        stage = work.tile([P, MH + MO], dt)
        nc.vector.tensor_copy(stage[:, 0:MH], par_ps[:, :])
        nc.vector.tensor_copy(stage[:, MH:MH + MO], out_ps[:, :])
        nc.sync.dma_start(ar3_i[:, :], stage[:, :])
        nc.gpsimd.collective_compute(
            "AllReduce", A.add, replica_groups=RG,
            ins=[ar3_i[:, :].opt()], outs=[ar3_o[:, :].opt()])
        pp = work.tile([P, MH + MO], dt)
        nc.sync.dma_start(pp[:, :], ar3_o[:, :])
        nc.vector.tensor_add(pp[:, 0:MH], pp[:, 0:MH], hb)
        nc.vector.tensor_add(pp[:, MH:MH + MO], pp[:, MH:MH + MO], ob)
        nc.sync.dma_start(d_oout[:, :], pp[:, MH:MH + MO])

        # erase/add slices for this core (params cols 8+pid / 16+pid).
        # Transposed rows must land on engine-addressable partitions (0, 32).
        ea2 = work.tile([P, 64], dt)
        nc.vector.memset(ea2[:, :], 0.0)
        nc.vector.tensor_copy(ea2[:, 0:1], pp[:, bass.ds(pid + 8, 1)])
        nc.vector.tensor_copy(ea2[:, 32:33], pp[:, bass.ds(pid + 16, 1)])
        nc.tensor.transpose(ea_ps[0:64, :], ea2[:, :], ident)
        er_row = work.tile([1, P], dt)
        ad_row = work.tile([1, P], dt)
        nc.scalar.activation(er_row[:, :], ea_ps[0:1, :], F.Sigmoid)
        nc.scalar.activation(ad_row[:, :], ea_ps[32:33, :], F.Tanh)
        er_bc = work.tile([P, P], dt)
        ad_bc = work.tile([P, P], dt)
        nc.gpsimd.partition_broadcast(er_bc[:, :], er_row[:, :])
        nc.gpsimd.partition_broadcast(ad_bc[:, :], ad_row[:, :])

        # write gate scalar
        wg_s = work.tile([1, 1], dt)
        nc.scalar.activation(wg_s[:, :], pp[0:1, MH - 1:MH], F.Sigmoid)
        wg_c = work.tile([P, 1], dt)
        nc.gpsimd.partition_broadcast(wg_c[:, :], wg_s[:, :])

        # softmax over write logits (params cols 24..55) * write_gate
        lg = pp[:, 24:56]
        wmx = work.tile([P, 1], dt)
        nc.vector.tensor_reduce(wmx[:, :], lg, axis=mybir.AxisListType.X,
                                op=A.max)
        wmxa = work.tile([P, 1], dt)
        nc.gpsimd.partition_all_reduce(wmxa[:, :], wmx[:, :], 128, ROp.max)
        wnmx = work.tile([P, 1], dt)
        nc.vector.tensor_scalar_mul(wnmx[:, :], wmxa[:, :], -1.0)
        wex = work.tile([P, SB], dt)
        wden_p = work.tile([P, 1], dt)
        nc.scalar.activation(wex[:, :], lg, F.Exp, bias=wnmx,
                             accum_out=wden_p[:, :])
        wden = work.tile([P, 1], dt)
        nc.gpsimd.partition_all_reduce(wden[:, :], wden_p[:, :], 128, ROp.add)
        wrd = work.tile([P, 1], dt)
        nc.vector.reciprocal(wrd[:, :], wden[:, :])
        w_t = work.tile([P, SB], dt)
        nc.vector.tensor_scalar(w_t[:, :], wex[:, :], wrd[:, :], wg_c[:, :],
                                op0=A.mult, op1=A.mult)

        # ============ memory update (local cols), DVE + GpSimd split ======
        for g in range(SB // 4):
            q3 = upd.tile([P, 4, P], dt, tag="q3", name=f"q3_{g}", bufs=2)
            for j in range(4):
                sb = g * 4 + j
                eng = nc.vector
                q1 = upd.tile([P, P], dt, tag="q1", name=f"q1_{sb}")
                eng.scalar_tensor_tensor(
                    q1[:, :], er_bc[:, :], w_t[:, sb:sb + 1], memF[:, sb, :],
                    op0=A.mult, op1=A.mult)
                q2 = upd.tile([P, P], dt, tag="q2", name=f"q2_{sb}")
                eng.tensor_sub(q2[:, :], memF[:, sb, :], q1[:, :])
                eng.scalar_tensor_tensor(
                    q3[:, j, :], ad_bc[:, :], w_t[:, sb:sb + 1], q2[:, :],
                    op0=A.mult, op1=A.add)
            nc.sync.dma_start(d_onm[:, g * 4:(g + 1) * 4, :], q3[:, :, :])

    nc.compile()
    _CACHE["nc"] = nc
    return nc


def _prep_inputs(inputs):
    """Full numpy inputs -> per-core in_maps."""
    import ml_dtypes
    x = np.asarray(inputs["x"], np.float32)
    h_prev = np.asarray(inputs["h_prev"], np.float32)
    c_prev = np.asarray(inputs["c_prev"], np.float32)
    memory = np.asarray(inputs["memory"], np.float32)
    usage = np.asarray(inputs["usage"], np.float32)
    read_W = np.asarray(inputs["read_W"], np.float32)
    read_b = np.asarray(inputs["read_b"], np.float32)
    W_ih = np.asarray(inputs["W_ih"], np.float32)
    b_ih = np.asarray(inputs["b_ih"], np.float32)
    W_hh = np.asarray(inputs["W_hh"], np.float32)
    b_hh = np.asarray(inputs["b_hh"], np.float32)
    head_W = np.asarray(inputs["head_W"], np.float32)
    head_b = np.asarray(inputs["head_b"], np.float32)
    out_W = np.asarray(inputs["out_W"], np.float32)
    out_b = np.asarray(inputs["out_b"], np.float32)

    gbias = _cm(b_ih + b_hh)                       # [128, 64]
    hb_pad = np.zeros(HEAD_PAD, np.float32)
    hb_pad[:HEAD] = head_b
    hbias = _cm(hb_pad)                            # [128, 57]
    obias = _cm(out_b)                             # [128, 16]
    hk_cm = _cm(h_prev[0])                         # [128, 16]
    cp_cm = _cm(c_prev[0])                         # [128, 16]
    us_cm = _cm(usage)                             # [128, 32]

    head_pad = np.zeros((HEAD_PAD, H), np.float32)
    head_pad[:HEAD] = head_W

    in_maps = []
    for c in range(C):
        sm = np.zeros((P, NS), np.float32)
        sm[:, _S_HK:_S_HK + KH] = hk_cm
        sm[:, _S_RB] = read_b[P * c:P * (c + 1)]
        sm[:, _S_GB:_S_GB + MG] = gbias
        sm[:, _S_HB:_S_HB + MH] = hbias
        sm[:, _S_OB:_S_OB + MO] = obias
        sm[:, _S_CP:_S_CP + MO] = cp_cm
        sm[:, _S_US:_S_US + SB] = us_cm
        sm[:, _S_ID:_S_ID + P] = np.eye(P, dtype=np.float32)

        smb = np.zeros((P, 8), ml_dtypes.bfloat16)
        xh, xl = _hilo(x[0, XC * c:XC * (c + 1)].reshape(2, P).T)
        hh, hl = _hilo(h_prev[0, HC * c:HC * (c + 1)].reshape(2, P).T)
        smb[:, 0:2] = xh
        smb[:, 2:4] = xl
        smb[:, 4:6] = hh
        smb[:, 6:8] = hl

        wihx_h, wihx_l = _pe_hilo(W_ih[:, XC * c:XC * (c + 1)])
        whh_h, whh_l = _pe_hilo(W_hh[:, HC * c:HC * (c + 1)])
        wihr_h, wihr_l = _pe_hilo(W_ih[:, IN + MC * c:IN + MC * (c + 1)])
        outwh_h, outwh_l = _pe_hilo(out_W[:, HC * c:HC * (c + 1)])
        outwr_h, outwr_l = _pe_hilo(out_W[:, H + MC * c:H + MC * (c + 1)])
        headw_b = _pe_layout(
            head_pad[:, HC * c:HC * (c + 1)]).astype(ml_dtypes.bfloat16)

        mem_c = memory[:, MC * c:MC * (c + 1)]      # [4096, 128]
        memn_h, memn_l = _hilo(np.ascontiguousarray(
            mem_c.reshape(SB, P, P).transpose(1, 0, 2)))
        in_maps.append({
            "wihx_hi": wihx_h, "wihx_lo": wihx_l,
            "whh_hi": whh_h, "whh_lo": whh_l,
            "wihr_hi": wihr_h[:, 0], "wihr_lo": wihr_l[:, 0],
            "headw": headw_b,
            "outwh_hi": outwh_h, "outwh_lo": outwh_l,
            "outwr_hi": outwr_h[:, 0], "outwr_lo": outwr_l[:, 0],
            "rwt": _pe_layout(read_W[P * c:P * (c + 1), :])[:, :, 0],
            "memT": np.ascontiguousarray(mem_c.T),
            "memN_hi": memn_h, "memN_lo": memn_l,
            "smalls": sm,
            "smallsb": smb,
        })
    return in_maps


def kernel(**inputs):
    from concourse.bass_utils import run_bass_kernel_spmd

    nc = _build()
    in_maps = _prep_inputs(inputs)
    res = run_bass_kernel_spmd(nc, in_maps, core_ids=list(range(C)))
    r0 = res.results[0]

    out = r0["o_out"].T.reshape(1, OUT).copy()
    h_new = r0["o_h"].T.reshape(1, H).copy()
    c_new = r0["o_c"].T.reshape(1, H).copy()
    usage_new = r0["o_usage"].T.reshape(S).copy()
    new_memory = np.empty((S, M), np.float32)
    for c in range(C):
        blk = res.results[c]["o_new_mem"]           # [128, 32, 128]
        new_memory[:, MC * c:MC * (c + 1)] = (
            blk.transpose(1, 0, 2).reshape(S, P))
    return (out, h_new, c_new, new_memory, usage_new)


# revision 21
# speedup vs baseline: 1.1277x; 1.1277x over previous
"""Trainium2 Bass kernel for nn_PrefrontalCortex (scatter_memory).

Sharding (8 cores, tensor-parallel / reduction-dim sharding):
  - read_W row-sharded: core c computes read_key[128c:128c+128] locally (fp32).
  - memory column-sharded (MEM_DIM): core c holds memory[:, 128c:128c+128]
    in two layouts (transposed for sim, natural for read_vec/update), fp32.
    sim partial = mem_shard @ read_key_slice  -> AllReduce #1 (16 KB).
    softmax + usage_new computed fully on every core; read_vec slice local.
  - W_ih / W_hh / out_W column-sharded (contraction dim), stored as bf16
    hi/lo pairs; GEMVs run 3 products (Whi*xhi + Whi*xlo + Wlo*xhi) into
    fp32 PSUM for near-fp32 accuracy at bf16 PE speed. head_W is plain bf16
    (its outputs - erase/add/write-logits - are insensitive).
    Partial GEMV outputs -> AllReduce #2 (gates, 32 KB) and
    AllReduce #3 (head params + out merged, 37 KB).
  - memory erase/add update is local per column shard; new_memory written
    sharded, gathered on host.

GEMVs use stationary 128x128 W^T blocks on the PE (outputs land col-major
across partitions). DMA rings: SP ring streams all weights in priority
order; ACT ring carries only latency-critical small transfers (collective
bounces, small outputs).
"""

import numpy as np

C = 8                       # cores
P = 128
H = 2048                    # hidden
IN = 2048                   # input dim
S = 4096                    # mem slots
M = 1024                    # mem dim
OUT = 2048
G4 = 4 * H                  # 8192
HEAD = 3 * M + S + 1        # 7169
MH = 57                     # head m-chunks (7296 padded)
HEAD_PAD = MH * P           # 7296
MG = G4 // P                # 64
MO = OUT // P               # 16
SB = S // P                 # 32
KH = H // P                 # 16  (h k-tiles, full)
XC = IN // C                # 256 per-core x cols (2 k-tiles)
HC = H // C                 # 256 per-core h cols
MC = M // C                 # 128 per-core mem cols (1 k-tile)

# smalls column layout (fp32)
_S_HK = 0          # [16] h_prev full, col-major (fp32, for read_key GEMV)
_S_RB = 16         # [1]  read_b slice
_S_GB = 17         # [64] b_ih + b_hh, col-major
_S_HB = 81         # [57] head_b padded, col-major
_S_OB = 138        # [16] out_b col-major
_S_CP = 154        # [16] c_prev col-major
_S_US = 170        # [32] usage col-major (slot)
_S_ID = 202        # [128] identity
NS = 330

_CACHE = {}


def _pe_layout(w):
    """w: [rows_out, cols_k] (rows % 128 == 0, cols % 128 == 0)
    -> [128, nk, nm, 128] with arr[p, k, m, j] = w[m*128 + j, k*128 + p]."""
    nm = w.shape[0] // P
    nk = w.shape[1] // P
    return np.ascontiguousarray(
        w.reshape(nm, P, nk, P).transpose(3, 2, 0, 1)
    ).astype(np.float32)


def _cm(v):
    """vector [n*128] -> col-major [128, n]: out[p, m] = v[m*128 + p]"""
    n = v.shape[0] // P
    return np.ascontiguousarray(v.reshape(n, P).T).astype(np.float32)


def _hilo(a):
    """f32 array -> (hi, lo) bf16 pair with hi + lo ~= a."""
    import ml_dtypes
    hi = a.astype(ml_dtypes.bfloat16)
    lo = (a - hi.astype(np.float32)).astype(ml_dtypes.bfloat16)
    return hi, lo


def _pe_hilo(w):
    return _hilo(_pe_layout(w))


def _build():
    if "nc" in _CACHE:
        return _CACHE["nc"]

    import concourse.bass as bass
    import concourse.bacc as bacc
    import concourse.mybir as mybir
    import concourse.tile as tile
    import bass_rust

    dt = mybir.dt.float32
    bf = mybir.dt.bfloat16
    A = mybir.AluOpType
    F = mybir.ActivationFunctionType
    ROp = bass_rust.ReduceOp

    nc = bacc.Bacc("TRN2", target_bir_lowering=False, debug=False, num_devices=C)

    # ---- I/O ----
    d_wihx = [nc.dram_tensor(f"wihx_{s}", [P, 2, MG, P], bf,
                             kind="ExternalInput") for s in ("hi", "lo")]
    d_whh = [nc.dram_tensor(f"whh_{s}", [P, 2, MG, P], bf,
                            kind="ExternalInput") for s in ("hi", "lo")]
    d_wihr = [nc.dram_tensor(f"wihr_{s}", [P, MG, P], bf,
                             kind="ExternalInput") for s in ("hi", "lo")]
    d_headw = nc.dram_tensor("headw", [P, 2, MH, P], bf, kind="ExternalInput")
    d_outwh = [nc.dram_tensor(f"outwh_{s}", [P, 2, MO, P], bf,
                              kind="ExternalInput") for s in ("hi", "lo")]
    d_outwr = [nc.dram_tensor(f"outwr_{s}", [P, MO, P], bf,
                              kind="ExternalInput") for s in ("hi", "lo")]
    d_rwt = nc.dram_tensor("rwt", [P, KH, P], dt, kind="ExternalInput")
    d_memT = nc.dram_tensor("memT", [P, S], dt, kind="ExternalInput")
    d_memN = [nc.dram_tensor(f"memN_{s}", [P, SB, P], bf,
                             kind="ExternalInput") for s in ("hi", "lo")]
    d_smalls = nc.dram_tensor("smalls", [P, NS], dt, kind="ExternalInput")
    d_smallsb = nc.dram_tensor("smallsb", [P, 8], bf, kind="ExternalInput")

    d_onm = nc.dram_tensor("o_new_mem", [P, SB, P], dt, kind="ExternalOutput")
    d_ous = nc.dram_tensor("o_usage", [P, SB], dt, kind="ExternalOutput")
    d_oout = nc.dram_tensor("o_out", [P, MO], dt, kind="ExternalOutput")
    d_oh = nc.dram_tensor("o_h", [P, MO], dt, kind="ExternalOutput")
    d_oc = nc.dram_tensor("o_c", [P, MO], dt, kind="ExternalOutput")

    RG = [list(range(C))]

    from contextlib import ExitStack

    with tile.TileContext(nc) as tc, ExitStack() as ctx:
        big = ctx.enter_context(tc.tile_pool(name="big", bufs=1))
        stream = ctx.enter_context(tc.tile_pool(name="stream", bufs=4))
        work = ctx.enter_context(tc.tile_pool(name="work", bufs=1))
        upd = ctx.enter_context(tc.tile_pool(name="upd", bufs=4))
        ps = ctx.enter_context(tc.tile_pool(name="ps", bufs=1, space="PSUM"))
        dram = ctx.enter_context(tc.tile_pool(name="dram", bufs=1, space="DRAM"))

        # ============ ACT ring: tiny, latency-critical ============
        smalls = work.tile([P, NS], dt)
        nc.scalar.dma_start(smalls[:, :], d_smalls[:, :])
        smallsb = work.tile([P, 8], bf)
        nc.scalar.dma_start(smallsb[:, :], d_smallsb[:, :])

        # read-path tensors on the ACT ring so they beat the weight flood
        rwt = big.tile([P, KH, P], dt)
        nc.scalar.dma_start(rwt[:, :, :], d_rwt[:, :, :])
        # ============ SP ring: all weights, in priority order ============
        memT = big.tile([P, S], dt)
        nc.sync.dma_start(memT[:, :], d_memT[:, :])
        memN = [big.tile([P, SB, P], bf, name=f"memN_{s}") for s in (0, 1)]
        for s in (0, 1):
            nc.sync.dma_start(memN[s][:, :, :], d_memN[s][:, :, :])
        memF = big.tile([P, SB, P], dt)
        wihr = [big.tile([P, MG, P], bf, name=f"wihr_{s}") for s in (0, 1)]
        for s in (0, 1):
            nc.sync.dma_start(wihr[s][:, :, :], d_wihr[s][:, :, :])
        outwr = [big.tile([P, MO, P], bf, name=f"outwr_{s}") for s in (0, 1)]
        for s in (0, 1):
            nc.sync.dma_start(outwr[s][:, :, :], d_outwr[s][:, :, :])

        # streamed gates weights: per m-half, per k-tile, hi then lo
        chunks = []
        chunk_src = []  # (dram_pair, k, hi/lo)
        for k in (0, 1):
            for s in (0, 1):
                chunk_src.append((d_wihx, k, s))
        for k in (0, 1):
            for s in (0, 1):
                chunk_src.append((d_whh, k, s))
        for mh in (0, 1):
            for (srcp, k, s) in chunk_src:
                t = stream.tile([P, 32, P], bf, tag="wch",
                                name=f"wch_{mh}_{k}_{s}_{srcp[s].name}")
                nc.sync.dma_start(t[:, :, :],
                                  srcp[s][:, k, mh * 32:(mh + 1) * 32, :])
                chunks.append(t)

        headw = big.tile([P, 2, MH, P], bf)
        nc.sync.dma_start(headw[:, 0, :, :], d_headw[:, 0, :, :])
        nc.sync.dma_start(headw[:, 1, :, :], d_headw[:, 1, :, :])
        outwh = [big.tile([P, 2, MO, P], bf, name=f"outwh_{s}")
                 for s in (0, 1)]
        for s in (0, 1):
            nc.sync.dma_start(outwh[s][:, :, :, :], d_outwh[s][:, :, :, :])

        # smalls views
        hk = smalls[:, _S_HK:_S_HK + KH]
        rb = smalls[:, _S_RB:_S_RB + 1]
        gb = smalls[:, _S_GB:_S_GB + MG]
        hb = smalls[:, _S_HB:_S_HB + MH]
        ob = smalls[:, _S_OB:_S_OB + MO]
        cp = smalls[:, _S_CP:_S_CP + MO]
        usg = smalls[:, _S_US:_S_US + SB]
        ident = smalls[:, _S_ID:_S_ID + P]
        x2h = smallsb[:, 0:2]
        x2l = smallsb[:, 2:4]
        h2h = smallsb[:, 4:6]
        h2l = smallsb[:, 6:8]

        # PSUM tiles
        rk_ps = ps.tile([P, 1], dt)
        sim_ps = ps.tile([P, SB], dt)
        gates_ps = ps.tile([P, MG], dt)
        rv_ps = ps.tile([P, 1], dt)
        par_ps = ps.tile([P, MH], dt)
        out_ps = ps.tile([P, MO], dt)
        ea_ps = ps.tile([64, P], dt)

        # DRAM bounce buffers for collectives
        ar1_i = dram.tile([P, SB], dt)
        ar1_o = dram.tile([P, SB], dt)
        ar2_i = dram.tile([P, MG], dt)
        ar2_o = dram.tile([P, MG], dt)
        ar3_i = dram.tile([P, MH + MO], dt)
        ar3_o = dram.tile([P, MH + MO], dt)

        MM = nc.tensor.matmul
        pid = nc.partition_id(engines=(mybir.EngineType.DVE,))

        # ============ read path (fp32) ============
        for k in range(KH):
            MM(rk_ps[:, 0:1], rwt[:, k, :], hk[:, k:k + 1],
               start=(k == 0), stop=(k == KH - 1), skip_group_check=True)
        rk = work.tile([P, 1], dt)
        nc.scalar.activation(rk[:, :], rk_ps[:, :], F.Tanh, bias=rb)

        for sb in range(SB):
            MM(sim_ps[:, sb:sb + 1], memT[:, sb * P:(sb + 1) * P], rk[:, 0:1],
               start=(sb == 0), stop=True, skip_group_check=True)
        sim_sb = work.tile([P, SB], dt)
        nc.vector.tensor_copy(sim_sb[:, :], sim_ps[:, :])
        nc.scalar.dma_start(ar1_i[:, :], sim_sb[:, :])
        nc.gpsimd.collective_compute(
            "AllReduce", A.add, replica_groups=RG,
            ins=[ar1_i[:, :].opt()], outs=[ar1_o[:, :].opt()])
        simf = work.tile([P, SB], dt)
        nc.scalar.dma_start(simf[:, :], ar1_o[:, :])

        # softmax(simf) -> rw [P, SB] col-major over slots
        mx = work.tile([P, 1], dt)
        nc.vector.tensor_reduce(mx[:, :], simf[:, :],
                                axis=mybir.AxisListType.X, op=A.max)
        mxa = work.tile([P, 1], dt)
        nc.gpsimd.partition_all_reduce(mxa[:, :], mx[:, :], 128, ROp.max)
        nmx = work.tile([P, 1], dt)
        nc.vector.tensor_scalar_mul(nmx[:, :], mxa[:, :], -1.0)
        ex = work.tile([P, SB], dt)
        den_p = work.tile([P, 1], dt)
        nc.scalar.activation(ex[:, :], simf[:, :], F.Exp, bias=nmx,
                             accum_out=den_p[:, :])
        den = work.tile([P, 1], dt)
        nc.gpsimd.partition_all_reduce(den[:, :], den_p[:, :], 128, ROp.add)
        rden = work.tile([P, 1], dt)
        nc.vector.reciprocal(rden[:, :], den[:, :])
        rw = work.tile([P, SB], dt)
        nc.vector.tensor_scalar_mul(rw[:, :], ex[:, :], rden[:, :])

        # usage_new (full, identical on every core)
        un = work.tile([P, SB], dt)
        nc.vector.tensor_add(un[:, :], usg, rw[:, :])
        nc.sync.dma_start(d_ous[:, :], un[:, :])
        mask01 = work.tile([P, SB], dt)
        nc.vector.tensor_scalar(mask01[:, :], un[:, :], 0.1, None, op0=A.is_ge)

        # ============ gates GEMV xh part (bf16 hi/lo partials) ============
        for mh in (0, 1):
            for i, (srcp, k, s) in enumerate(chunk_src):
                t = chunks[mh * 8 + i]
                rh, rl = (x2h, x2l) if srcp is d_wihx else (h2h, h2l)
                for m in range(32):
                    gm = mh * 32 + m
                    last_gates_mm = MM(
                        gates_ps[:, gm:gm + 1], t[:, m, :], rh[:, k:k + 1],
                        start=(mh == 0 and i == 0 and m == 0), stop=False,
                        skip_group_check=True)
                    if s == 0:  # hi weights also multiply lo rhs
                        last_gates_mm = MM(
                            gates_ps[:, gm:gm + 1], t[:, m, :], rl[:, k:k + 1],
                            start=False, stop=False, skip_group_check=True)

        # reconstruct f32 memory for the update (hi + lo), off-chain
        for sb in range(SB):
            nc.vector.tensor_add(memF[:, sb, :], memN[0][:, sb, :],
                                 memN[1][:, sb, :])

        # rw hi/lo split for the bf16 read_vec GEMV
        rw_h = work.tile([P, SB], bf)
        nc.vector.tensor_copy(rw_h[:, :], rw[:, :])
        rw_hf = work.tile([P, SB], dt)
        nc.vector.tensor_copy(rw_hf[:, :], rw_h[:, :])
        rw_lf = work.tile([P, SB], dt)
        nc.vector.tensor_sub(rw_lf[:, :], rw[:, :], rw_hf[:, :])
        rw_l = work.tile([P, SB], bf)
        nc.vector.tensor_copy(rw_l[:, :], rw_lf[:, :])

        # ============ read_vec (bf16 hi/lo, after AR1) ============
        from concourse.tile_rust import add_dep_helper
        first_rv = None
        for sb in range(SB):
            i1 = MM(rv_ps[:, 0:1], memN[0][:, sb, :], rw_h[:, sb:sb + 1],
                    start=(sb == 0), stop=False, skip_group_check=True)
            if first_rv is None:
                first_rv = i1
                add_dep_helper(first_rv.ins, last_gates_mm.ins, sync=True,
                               reason="keep rv after streamed gates-xh")
            MM(rv_ps[:, 0:1], memN[0][:, sb, :], rw_l[:, sb:sb + 1],
               start=False, stop=False, skip_group_check=True)
            MM(rv_ps[:, 0:1], memN[1][:, sb, :], rw_h[:, sb:sb + 1],
               start=False, stop=(sb == SB - 1), skip_group_check=True)
        rv = work.tile([P, 1], dt)
        nc.vector.tensor_copy(rv[:, :], rv_ps[:, :])
        rv_h = work.tile([P, 1], bf)
        nc.vector.tensor_copy(rv_h[:, :], rv[:, :])
        rv_hf = work.tile([P, 1], dt)
        nc.vector.tensor_copy(rv_hf[:, :], rv_h[:, :])
        rv_lf = work.tile([P, 1], dt)
        nc.vector.tensor_sub(rv_lf[:, :], rv[:, :], rv_hf[:, :])
        rv_l = work.tile([P, 1], bf)
        nc.vector.tensor_copy(rv_l[:, :], rv_lf[:, :])

        # mask reconstructed memory in place (DVE; keep the ACT ring free)
        for sb in range(SB):
            nc.vector.tensor_scalar_mul(memF[:, sb, :], memF[:, sb, :],
                                        mask01[:, sb:sb + 1])

        # read_vec contribution closes the gates accumulation
        for m in range(MG):
            MM(gates_ps[:, m:m + 1], wihr[0][:, m, :], rv_h[:, 0:1],
               start=False, stop=False, skip_group_check=True)
            MM(gates_ps[:, m:m + 1], wihr[0][:, m, :], rv_l[:, 0:1],
               start=False, stop=False, skip_group_check=True)
            MM(gates_ps[:, m:m + 1], wihr[1][:, m, :], rv_h[:, 0:1],
               start=False, stop=(m == MG - 1), skip_group_check=True)

        gpart = work.tile([P, MG], dt)
        nc.vector.tensor_copy(gpart[:, :], gates_ps[:, :])
        nc.sync.dma_start(ar2_i[:, :], gpart[:, :])
        nc.gpsimd.collective_compute(
            "AllReduce", A.add, replica_groups=RG,
            ins=[ar2_i[:, :].opt()], outs=[ar2_o[:, :].opt()])
        gts = work.tile([P, MG], dt)
        nc.sync.dma_start(gts[:, :], ar2_o[:, :])
        nc.vector.tensor_add(gts[:, :], gts[:, :], gb)

        # out: rv part can start as soon as rv is ready
        for m in range(MO):
            MM(out_ps[:, m:m + 1], outwr[0][:, m, :], rv_h[:, 0:1],
               start=(m == 0), stop=False, skip_group_check=True)
            MM(out_ps[:, m:m + 1], outwr[0][:, m, :], rv_l[:, 0:1],
               start=False, stop=False, skip_group_check=True)
            MM(out_ps[:, m:m + 1], outwr[1][:, m, :], rv_h[:, 0:1],
               start=False, stop=False, skip_group_check=True)

        # ============ LSTM cell (full, identical on every core) ============
        gi = gts[:, 0:16]
        gf = gts[:, 16:32]
        gg = gts[:, 32:48]
        go = gts[:, 48:64]
        si = work.tile([P, MO], dt)
        sf = work.tile([P, MO], dt)
        tg = work.tile([P, MO], dt)
        so = work.tile([P, MO], dt)
        nc.scalar.activation(si[:, :], gi, F.Sigmoid)
        nc.scalar.activation(sf[:, :], gf, F.Sigmoid)
        nc.scalar.activation(so[:, :], go, F.Sigmoid)
        nc.scalar.activation(tg[:, :], gg, F.Tanh)
        t1 = work.tile([P, MO], dt)
        nc.vector.tensor_mul(t1[:, :], si[:, :], tg[:, :])
        t2 = work.tile([P, MO], dt)
        nc.vector.tensor_mul(t2[:, :], sf[:, :], cp)
        c_t = work.tile([P, MO], dt)
        nc.vector.tensor_add(c_t[:, :], t1[:, :], t2[:, :])
        tc_ = work.tile([P, MO], dt)
        nc.scalar.activation(tc_[:, :], c_t[:, :], F.Tanh)
        h_t = work.tile([P, MO], dt)
        nc.vector.tensor_mul(h_t[:, :], so[:, :], tc_[:, :])
        nc.sync.dma_start(d_oc[:, :], c_t[:, :])
        nc.sync.dma_start(d_oh[:, :], h_t[:, :])

        # this core's two h k-tiles (dynamic by core id), bf16 hi/lo
        hloc = work.tile([P, 2], dt)
        nc.vector.tensor_copy(hloc[:, :], h_t[:, bass.ts(pid, 2)])
        hloc_h = work.tile([P, 2], bf)
        nc.vector.tensor_copy(hloc_h[:, :], hloc[:, :])
        hloc_hf = work.tile([P, 2], dt)
        nc.vector.tensor_copy(hloc_hf[:, :], hloc_h[:, :])
        hloc_lf = work.tile([P, 2], dt)
        nc.vector.tensor_sub(hloc_lf[:, :], hloc[:, :], hloc_hf[:, :])
        hloc_l = work.tile([P, 2], bf)
        nc.vector.tensor_copy(hloc_l[:, :], hloc_lf[:, :])

        # ============ head + out h-part GEMVs (partial) ============
        for kk in (0, 1):
            for m in range(MO):
                MM(out_ps[:, m:m + 1], outwh[0][:, kk, m, :],
                   hloc_h[:, kk:kk + 1],
                   start=False, stop=False, skip_group_check=True)
                MM(out_ps[:, m:m + 1], outwh[0][:, kk, m, :],
                   hloc_l[:, kk:kk + 1],
                   start=False, stop=False, skip_group_check=True)
                MM(out_ps[:, m:m + 1], outwh[1][:, kk, m, :],
                   hloc_h[:, kk:kk + 1],
                   start=False, stop=(kk == 1 and m == MO - 1),
                   skip_group_check=True)
        for kk in (0, 1):
            for m in range(MH):
                MM(par_ps[:, m:m + 1], headw[:, kk, m, :],
                   hloc_h[:, kk:kk + 1],
                   start=(kk == 0 and m == 0), stop=False,
                   skip_group_check=True)
                MM(par_ps[:, m:m + 1], headw[:, kk, m, :],
                   hloc_l[:, kk:kk + 1],
                   start=False, stop=(kk == 1 and m == MH - 1),
                   skip_group_check=True)

        stage = work.tile([P, MH + MO], dt)
        nc.vector.tensor_copy(stage[:, 0:MH], par_ps[:, :])
        nc.vector.tensor_copy(stage[:, MH:MH + MO], out_ps[:, :])
        nc.sync.dma_start(ar3_i[:, :], stage[:, :])
        nc.gpsimd.collective_compute(
            "AllReduce", A.add, replica_groups=RG,
            ins=[ar3_i[:, :].opt()], outs=[ar3_o[:, :].opt()])
        pp = work.tile([P, MH + MO], dt)
        nc.sync.dma_start(pp[:, :], ar3_o[:, :])
        nc.vector.tensor_add(pp[:, 0:MH], pp[:, 0:MH], hb)
        nc.vector.tensor_add(pp[:, MH:MH + MO], pp[:, MH:MH + MO], ob)
        nc.sync.dma_start(d_oout[:, :], pp[:, MH:MH + MO])

        # erase/add slices for this core (params cols 8+pid / 16+pid).
        # Transposed rows must land on engine-addressable partitions (0, 32).
        ea2 = work.tile([P, 64], dt)
        nc.vector.memset(ea2[:, :], 0.0)
        nc.vector.tensor_copy(ea2[:, 0:1], pp[:, bass.ds(pid + 8, 1)])
        nc.vector.tensor_copy(ea2[:, 32:33], pp[:, bass.ds(pid + 16, 1)])
        nc.tensor.transpose(ea_ps[0:64, :], ea2[:, :], ident)
        er_row = work.tile([1, P], dt)
        ad_row = work.tile([1, P], dt)
        nc.scalar.activation(er_row[:, :], ea_ps[0:1, :], F.Sigmoid)
        nc.scalar.activation(ad_row[:, :], ea_ps[32:33, :], F.Tanh)
        er_bc = work.tile([P, P], dt)
        ad_bc = work.tile([P, P], dt)
        nc.gpsimd.partition_broadcast(er_bc[:, :], er_row[:, :])
        nc.gpsimd.partition_broadcast(ad_bc[:, :], ad_row[:, :])

        # write gate scalar
        wg_s = work.tile([1, 1], dt)
        nc.scalar.activation(wg_s[:, :], pp[0:1, MH - 1:MH], F.Sigmoid)
        wg_c = work.tile([P, 1], dt)
        nc.gpsimd.partition_broadcast(wg_c[:, :], wg_s[:, :])

        # softmax over write logits (params cols 24..55) * write_gate
        lg = pp[:, 24:56]
        wmx = work.tile([P, 1], dt)
        nc.vector.tensor_reduce(wmx[:, :], lg, axis=mybir.AxisListType.X,
                                op=A.max)
        wmxa = work.tile([P, 1], dt)
        nc.gpsimd.partition_all_reduce(wmxa[:, :], wmx[:, :], 128, ROp.max)
        wnmx = work.tile([P, 1], dt)
        nc.vector.tensor_scalar_mul(wnmx[:, :], wmxa[:, :], -1.0)
        wex = work.tile([P, SB], dt)
        wden_p = work.tile([P, 1], dt)
        nc.scalar.activation(wex[:, :], lg, F.Exp, bias=wnmx,
                             accum_out=wden_p[:, :])
        wden = work.tile([P, 1], dt)
        nc.gpsimd.partition_all_reduce(wden[:, :], wden_p[:, :], 128, ROp.add)
        wrd = work.tile([P, 1], dt)
        nc.vector.reciprocal(wrd[:, :], wden[:, :])
        w_t = work.tile([P, SB], dt)
        nc.vector.tensor_scalar(w_t[:, :], wex[:, :], wrd[:, :], wg_c[:, :],
                                op0=A.mult, op1=A.mult)

        # ============ memory update (local cols), DVE + GpSimd split ======
        for g in range(SB // 4):
            q3 = upd.tile([P, 4, P], dt, tag="q3", name=f"q3_{g}", bufs=2)
            for j in range(4):
                sb = g * 4 + j
                eng = nc.vector
                q1 = upd.tile([P, P], dt, tag="q1", name=f"q1_{sb}")
                eng.scalar_tensor_tensor(
                    q1[:, :], er_bc[:, :], w_t[:, sb:sb + 1], memF[:, sb, :],
                    op0=A.mult, op1=A.mult)
                q2 = upd.tile([P, P], dt, tag="q2", name=f"q2_{sb}")
                eng.tensor_sub(q2[:, :], memF[:, sb, :], q1[:, :])
                eng.scalar_tensor_tensor(
                    q3[:, j, :], ad_bc[:, :], w_t[:, sb:sb + 1], q2[:, :],
                    op0=A.mult, op1=A.add)
            nc.sync.dma_start(d_onm[:, g * 4:(g + 1) * 4, :], q3[:, :, :])

    nc.compile()
    _CACHE["nc"] = nc
    return nc


def _prep_inputs(inputs):
    """Full numpy inputs -> per-core in_maps."""
    import ml_dtypes
    x = np.asarray(inputs["x"], np.float32)
    h_prev = np.asarray(inputs["h_prev"], np.float32)
    c_prev = np.asarray(inputs["c_prev"], np.float32)
    memory = np.asarray(inputs["memory"], np.float32)
    usage = np.asarray(inputs["usage"], np.float32)
    read_W = np.asarray(inputs["read_W"], np.float32)
    read_b = np.asarray(inputs["read_b"], np.float32)
    W_ih = np.asarray(inputs["W_ih"], np.float32)
    b_ih = np.asarray(inputs["b_ih"], np.float32)
    W_hh = np.asarray(inputs["W_hh"], np.float32)
    b_hh = np.asarray(inputs["b_hh"], np.float32)
    head_W = np.asarray(inputs["head_W"], np.float32)
    head_b = np.asarray(inputs["head_b"], np.float32)
    out_W = np.asarray(inputs["out_W"], np.float32)
    out_b = np.asarray(inputs["out_b"], np.float32)

    gbias = _cm(b_ih + b_hh)                       # [128, 64]
    hb_pad = np.zeros(HEAD_PAD, np.float32)
    hb_pad[:HEAD] = head_b
    hbias = _cm(hb_pad)                            # [128, 57]
    obias = _cm(out_b)                             # [128, 16]
    hk_cm = _cm(h_prev[0])                         # [128, 16]
    cp_cm = _cm(c_prev[0])                         # [128, 16]
    us_cm = _cm(usage)                             # [128, 32]

    head_pad = np.zeros((HEAD_PAD, H), np.float32)
    head_pad[:HEAD] = head_W

    in_maps = []
    for c in range(C):
        sm = np.zeros((P, NS), np.float32)
        sm[:, _S_HK:_S_HK + KH] = hk_cm
        sm[:, _S_RB] = read_b[P * c:P * (c + 1)]
        sm[:, _S_GB:_S_GB + MG] = gbias
        sm[:, _S_HB:_S_HB + MH] = hbias
        sm[:, _S_OB:_S_OB + MO] = obias
        sm[:, _S_CP:_S_CP + MO] = cp_cm
        sm[:, _S_US:_S_US + SB] = us_cm
        sm[:, _S_ID:_S_ID + P] = np.eye(P, dtype=np.float32)

        smb = np.zeros((P, 8), ml_dtypes.bfloat16)
        xh, xl = _hilo(x[0, XC * c:XC * (c + 1)].reshape(2, P).T)
        hh, hl = _hilo(h_prev[0, HC * c:HC * (c + 1)].reshape(2, P).T)
        smb[:, 0:2] = xh
        smb[:, 2:4] = xl
        smb[:, 4:6] = hh
        smb[:, 6:8] = hl

        wihx_h, wihx_l = _pe_hilo(W_ih[:, XC * c:XC * (c + 1)])
        whh_h, whh_l = _pe_hilo(W_hh[:, HC * c:HC * (c + 1)])
        wihr_h, wihr_l = _pe_hilo(W_ih[:, IN + MC * c:IN + MC * (c + 1)])
        outwh_h, outwh_l = _pe_hilo(out_W[:, HC * c:HC * (c + 1)])
        outwr_h, outwr_l = _pe_hilo(out_W[:, H + MC * c:H + MC * (c + 1)])
        headw_b = _pe_layout(
            head_pad[:, HC * c:HC * (c + 1)]).astype(ml_dtypes.bfloat16)

        mem_c = memory[:, MC * c:MC * (c + 1)]      # [4096, 128]
        memn_h, memn_l = _hilo(np.ascontiguousarray(
            mem_c.reshape(SB, P, P).transpose(1, 0, 2)))
        in_maps.append({
            "wihx_hi": wihx_h, "wihx_lo": wihx_l,
            "whh_hi": whh_h, "whh_lo": whh_l,
            "wihr_hi": wihr_h[:, 0], "wihr_lo": wihr_l[:, 0],
            "headw": headw_b,
            "outwh_hi": outwh_h, "outwh_lo": outwh_l,
            "outwr_hi": outwr_h[:, 0], "outwr_lo": outwr_l[:, 0],
            "rwt": _pe_layout(read_W[P * c:P * (c + 1), :])[:, :, 0],
            "memT": np.ascontiguousarray(mem_c.T),
            "memN_hi": memn_h, "memN_lo": memn_l,
            "smalls": sm,
            "smallsb": smb,
        })
    return in_maps


def kernel(**inputs):
    from concourse.bass_utils import run_bass_kernel_spmd

    nc = _build()
    in_maps = _prep_inputs(inputs)
    res = run_bass_kernel_spmd(nc, in_maps, core_ids=list(range(C)))
    r0 = res.results[0]

    out = r0["o_out"].T.reshape(1, OUT).copy()
    h_new = r0["o_h"].T.reshape(1, H).copy()
    c_new = r0["o_c"].T.reshape(1, H).copy()
    usage_new = r0["o_usage"].T.reshape(S).copy()
    new_memory = np.empty((S, M), np.float32)
    for c in range(C):
        blk = res.results[c]["o_new_mem"]           # [128, 32, 128]
        new_memory[:, MC * c:MC * (c + 1)] = (
            blk.transpose(1, 0, 2).reshape(S, P))
    return (out, h_new, c_new, new_memory, usage_new)
